# revision 3
# baseline (speedup 1.0000x reference)
"""Trainium2 Bass kernel for nn_MultiHeadAttention (B=8, S=1024, D=1024, H=16).

Strategy: data-parallel over batch across the 8 NeuronCores (attention is
independent per batch element, so no collectives are needed). Each core runs an
identical Tile program on its batch slice:

  - QKV projections as bf16 matmuls (weights pre-concatenated per head on host,
    TEMPER folded into Wq; inputs fed pre-transposed so the contraction dim is
    on partitions).
  - Scores are computed twice per head - once natural (i on partitions) for the
    softmax/attns output, once transposed (j on partitions) to feed P@V without
    an on-chip transpose of P.
  - The boolean mask is applied as an additive -50 bias (via an
    identity-matmul accumulate into PSUM) on the natural path and as a {0,1}
    multiplier on the transposed path.
  - exp runs on the scalar engine with accum_out producing row sums for free;
    the softmax divide is a per-partition tensor_scalar; the transposed path
    stays unnormalized and the 1/r correction is applied to O = P@V via a
    DMA-broadcast of 1/r.
  - Output projection + residual + (unscaled) layernorm on-chip; the ln_g/ln_b
    affine is exact and applied on host (ones/zeros in practice).

Outputs: normalized attention probabilities per head (f32) and the layernormed
output (f32), gathered/reassembled on host.
"""

import numpy as np
import ml_dtypes

B, S, D = 8, 1024, 1024
H, DK, DV = 16, 64, 64
NT = S // 128
TEMPER = float(np.sqrt(D))
NCORES = 8
BF16 = ml_dtypes.bfloat16

# Set by test harness to capture a NTFF profile (requires the axon profile
# hook to be installed by the caller). Off by default for robustness.
PROFILE = False
LAST_EXEC_NS = None
LAST_RESULTS = None

_compiled_nc = None


def _build_nc():
    import concourse.bass as bass
    import concourse.mybir as mybir
    import concourse.tile as tile
    from concourse import bacc
    from concourse.masks import make_identity

    bf = mybir.dt.bfloat16
    f32 = mybir.dt.float32
    Exp = mybir.ActivationFunctionType.Exp
    Sqrt = mybir.ActivationFunctionType.Sqrt

    nc = bacc.Bacc("TRN2", target_bir_lowering=False, debug=False,
                   num_devices=NCORES)

    p_qt = nc.declare_dram_parameter("qt", [D, S], bf, isOutput=False)
    p_kt = nc.declare_dram_parameter("kt", [D, S], bf, isOutput=False)
    p_vt = nc.declare_dram_parameter("vt", [D, S], bf, isOutput=False)
    p_wq = nc.declare_dram_parameter("wq", [D, H * DK], bf, isOutput=False)
    p_wk = nc.declare_dram_parameter("wk", [D, H * DK], bf, isOutput=False)
    p_wv = nc.declare_dram_parameter("wv", [D, H * DV], bf, isOutput=False)
    p_pwt = nc.declare_dram_parameter("pwt", [H * DV, D], bf, isOutput=False)
    p_mb = nc.declare_dram_parameter("mb", [S, S], bf, isOutput=False)
    p_mft = nc.declare_dram_parameter("mft", [S, S], bf, isOutput=False)
    p_qres = nc.declare_dram_parameter("qres", [S, D], f32, isOutput=False)
    p_attn = nc.declare_dram_parameter("attns", [H, S, S], f32, isOutput=True)
    p_out = nc.declare_dram_parameter("outn", [S, D], f32, isOutput=True)
    rdram = nc.dram_tensor("rscratch", [NT, 2, S], f32)

    def t3(p):
        # DRAM (R, C) -> SBUF-shaped (128, R/128, C) view
        return p.ap().rearrange("(t p) c -> p t c", p=128)

    with tile.TileContext(nc) as tc:
        ident, _f0 = tc.tile([128, 128], bf, name="ident")
        make_identity(nc, ident)
        epst, _f1 = tc.tile([128, 1], f32, name="epst")
        nc.vector.memset(epst, 1e-5)
        QT, _f2 = tc.tile([128, NT, S], bf, name="QT")
        KT, _f3 = tc.tile([128, NT, S], bf, name="KT")
        VS, _f4 = tc.tile([128, NT, H * DV], bf, name="VS")
        MBs, _f5 = tc.tile([128, NT, S], bf, name="MBs")
        MFTs, _f6 = tc.tile([128, NT, S], bf, name="MFTs")
        OT, _f7 = tc.tile([128, NT, S], bf, name="OT")

        with tc.tile_pool(name="w2m", bufs=3) as wpool, \
             tc.tile_pool(name="pa", bufs=8) as papool, \
             tc.tile_pool(name="fx", bufs=2) as fxpool, \
             tc.tile_pool(name="sm", bufs=4) as smpool, \
             tc.tile_pool(name="psb", bufs=3, space="PSUM") as psb, \
             tc.tile_pool(name="pss", bufs=2, space="PSUM") as pss:

            nc.sync.dma_start(out=MBs, in_=t3(p_mb))
            nc.sync.dma_start(out=MFTs, in_=t3(p_mft))

            # ---- Stage P: projections ----
            # QT = (q @ Wq/TEMPER).T, KT = (k @ Wk).T  (head-dim on partitions)
            # VS = v @ Wv (natural: sequence on partitions)
            for src, wsrc, dst, transposed in ((p_qt, p_wq, QT, True),
                                               (p_kt, p_wk, KT, True),
                                               (p_vt, p_wv, VS, False)):
                x_t = wpool.tile([128, NT, S], bf, tag="w2m", name="x_t")
                w_t = wpool.tile([128, NT, S], bf, tag="w2m", name="w_t")
                nc.sync.dma_start(out=x_t, in_=t3(src))
                nc.sync.dma_start(out=w_t, in_=t3(wsrc))
                for m in range(NT):
                    ps = psb.tile([128, 1024], f32, tag="big", name="ps")
                    for nch in range(2):
                        nsl = slice(nch * 512, nch * 512 + 512)
                        for kt_i in range(NT):
                            if transposed:
                                lhs = w_t[:, kt_i, m * 128:(m + 1) * 128]
                                rhs = x_t[:, kt_i, nsl]
                            else:
                                lhs = x_t[:, kt_i, m * 128:(m + 1) * 128]
                                rhs = w_t[:, kt_i, nsl]
                            nc.tensor.matmul(ps[:, nsl], lhsT=lhs, rhs=rhs,
                                             start=(kt_i == 0),
                                             stop=(kt_i == NT - 1))
                    nc.vector.tensor_copy(out=dst[:, m, :], in_=ps)

            # ---- Stage A: attention, head pairs packed in the PE array ----
            for t2 in range(NT):
                puts = []
                rinvs = []
                for hh in range(2):
                    h = 2 * t2 + hh
                    hs = slice(hh * 64, hh * 64 + 64)
                    # path A: natural scores -> exp(+rowsum) -> normalize -> attns
                    racc = smpool.tile([128, NT], f32, tag="racc", name="racc")
                    pa_tiles = []
                    for it in range(NT):
                        psA = psb.tile([128, 1024], f32, tag="big", name="psA")
                        for jc in range(2):
                            jsl = slice(jc * 512, jc * 512 + 512)
                            nc.tensor.matmul(psA[:, jsl], lhsT=ident,
                                             rhs=MBs[:, it, jsl],
                                             start=True, stop=False)
                            nc.tensor.matmul(psA[:, jsl],
                                             lhsT=QT[hs, t2, it * 128:(it + 1) * 128],
                                             rhs=KT[hs, t2, jsl],
                                             start=False, stop=True,
                                             tile_position=(hh * 64, 0))
                        pa = papool.tile([128, 1024], bf, tag="pa", name="pa")
                        nc.scalar.activation(out=pa, in_=psA, func=Exp,
                                             accum_out=racc[:, it:it + 1])
                        pa_tiles.append(pa)
                    rinv = smpool.tile([128, NT], f32, tag="rinv", name="rinv")
                    nc.vector.reciprocal(out=rinv, in_=racc)
                    rinvs.append(rinv)
                    for it in range(NT):
                        nc.vector.tensor_scalar_mul(out=pa_tiles[it],
                                                    in0=pa_tiles[it],
                                                    scalar1=rinv[:, it:it + 1])
                        nc.gpsimd.dma_start(
                            out=p_attn.ap()[h, it * 128:(it + 1) * 128, :],
                            in_=pa_tiles[it])
                    # path B: transposed scores -> exp -> mask-mul (unnormalized)
                    put = wpool.tile([128, NT, S], bf, tag="w2m", name="put")
                    for jt in range(NT):
                        psB = psb.tile([128, 1024], f32, tag="big", name="psB")
                        for ic in range(2):
                            isl = slice(ic * 512, ic * 512 + 512)
                            nc.tensor.matmul(psB[:, isl],
                                             lhsT=KT[hs, t2, jt * 128:(jt + 1) * 128],
                                             rhs=QT[hs, t2, isl],
                                             start=True, stop=True,
                                             tile_position=(hh * 64, 0))
                        nc.scalar.activation(out=put[:, jt, :], in_=psB, func=Exp)
                        nc.vector.tensor_mul(out=put[:, jt, :],
                                             in0=put[:, jt, :],
                                             in1=MFTs[:, jt, :])
                    puts.append(put)
                # 1/r broadcast: write i-major to DRAM, read back replicated
                for hh in range(2):
                    nc.gpsimd.dma_start(
                        out=rdram.ap()[t2, hh].rearrange("(t p) -> p t", p=128),
                        in_=rinvs[hh])
                rrep = fxpool.tile([128, 1024], f32, tag="rrep", name="rrep")
                for hh in range(2):
                    src_ap = rdram.ap()[t2, hh]
                    b_ap = bass.AP(tensor=src_ap.tensor, offset=src_ap.offset,
                                   ap=[[0, 64]] + list(src_ap.ap))
                    nc.sync.dma_start(out=rrep[hh * 64:hh * 64 + 64, :], in_=b_ap)
                # P@V for the head pair (col-packed), then normalize rows by 1/r
                for ic in range(2):
                    isl = slice(ic * 512, ic * 512 + 512)
                    psO = pss.tile([128, 512], f32, tag="small", name="psO")
                    for hh in range(2):
                        h = 2 * t2 + hh
                        for jt in range(NT):
                            nc.tensor.matmul(psO[hh * 64:hh * 64 + 64, :],
                                             lhsT=VS[:, jt, h * 64:(h + 1) * 64],
                                             rhs=puts[hh][:, jt, isl],
                                             start=(jt == 0), stop=(jt == NT - 1),
                                             tile_position=(0, hh * 64))
                    nc.vector.tensor_mul(out=OT[:, t2, isl], in0=psO,
                                         in1=rrep[:, isl])

            # ---- Stage F: projection + residual + layernorm ----
            pwt_t = wpool.tile([128, NT, D], bf, tag="w2m", name="pwt_t")
            nc.sync.dma_start(out=pwt_t, in_=t3(p_pwt))
            for st in range(NT):
                qr = fxpool.tile([128, 1024], f32, tag="qr", name="qr")
                nc.sync.dma_start(out=qr, in_=p_qres.ap()[st * 128:(st + 1) * 128, :])
                xm = fxpool.tile([128, 1024], f32, tag="xm", name="xm")
                for dc in range(2):
                    dsl = slice(dc * 512, dc * 512 + 512)
                    psF = pss.tile([128, 512], f32, tag="small", name="psF")
                    for ct in range(NT):
                        nc.tensor.matmul(psF,
                                         lhsT=OT[:, ct, st * 128:(st + 1) * 128],
                                         rhs=pwt_t[:, ct, dsl],
                                         start=(ct == 0), stop=(ct == NT - 1))
                    nc.vector.tensor_add(out=xm[:, dsl], in0=psF, in1=qr[:, dsl])
                stats = smpool.tile([128, 2, 6], f32, tag="stats", name="stats")
                for c2 in range(2):
                    nc.vector.bn_stats(out=stats[:, c2, :],
                                       in_=xm[:, c2 * 512:(c2 + 1) * 512])
                mv = smpool.tile([128, 2], f32, tag="mv", name="mv")
                nc.vector.bn_aggr(out=mv, in_=stats)
                std = smpool.tile([128, 1], f32, tag="std", name="std")
                nc.scalar.activation(out=std, in_=mv[:, 1:2], func=Sqrt,
                                     bias=epst, scale=1.0)
                nc.vector.reciprocal(out=std, in_=std)
                nc.vector.tensor_scalar(out=xm, in0=xm,
                                        scalar1=mv[:, 0:1], scalar2=std,
                                        op0=mybir.AluOpType.subtract,
                                        op1=mybir.AluOpType.mult)
                nc.sync.dma_start(out=p_out.ap()[st * 128:(st + 1) * 128, :],
                                  in_=xm)
        for f in (_f7, _f6, _f5, _f4, _f3, _f2, _f1, _f0):
            f()
    nc.compile()
    return nc


def _get_nc():
    global _compiled_nc
    if _compiled_nc is None:
        _compiled_nc = _build_nc()
    return _compiled_nc


def kernel(q, k, v, attn_mask, w_qs, w_ks, w_vs, proj_w, proj_b, ln_g, ln_b):
    global LAST_EXEC_NS, LAST_RESULTS
    from concourse.bass_utils import run_bass_kernel_spmd

    q = np.asarray(q, np.float32)
    k = np.asarray(k, np.float32)
    v = np.asarray(v, np.float32)
    attn_mask = np.asarray(attn_mask, bool)
    w_qs = np.asarray(w_qs, np.float32)
    w_ks = np.asarray(w_ks, np.float32)
    w_vs = np.asarray(w_vs, np.float32)
    proj_w = np.asarray(proj_w, np.float32)
    proj_b = np.asarray(proj_b, np.float32)
    ln_g = np.asarray(ln_g, np.float32)
    ln_b = np.asarray(ln_b, np.float32)

    # host-side weight prep (shared across cores)
    wq = np.ascontiguousarray(
        (w_qs.transpose(1, 0, 2).reshape(D, H * DK) / TEMPER)).astype(BF16)
    wk = np.ascontiguousarray(
        w_ks.transpose(1, 0, 2).reshape(D, H * DK)).astype(BF16)
    wv = np.ascontiguousarray(
        w_vs.transpose(1, 0, 2).reshape(D, H * DV)).astype(BF16)
    pwt = np.ascontiguousarray(proj_w.T).astype(BF16)
    qres_all = q + proj_b  # fold the projection bias into the residual

    in_maps = []
    for b in range(B):
        in_maps.append(dict(
            qt=np.ascontiguousarray(q[b].T).astype(BF16),
            kt=np.ascontiguousarray(k[b].T).astype(BF16),
            vt=np.ascontiguousarray(v[b].T).astype(BF16),
            wq=wq, wk=wk, wv=wv, pwt=pwt,
            mb=np.where(attn_mask[b], np.float32(-50.0),
                        np.float32(0.0)).astype(BF16),
            mft=np.ascontiguousarray((~attn_mask[b]).T.astype(np.float32)).astype(BF16),
            qres=np.ascontiguousarray(qres_all[b]).astype(np.float32),
        ))

    nc = _get_nc()
    res = run_bass_kernel_spmd(nc, in_maps, core_ids=list(range(NCORES)),
                               trace=PROFILE)
    LAST_EXEC_NS = res.exec_time_ns
    LAST_RESULTS = res

    out = np.stack([res.results[b]["outn"] for b in range(B)], axis=0)
    out = out * ln_g + ln_b  # exact layernorm affine on host

    attns = np.empty((H * B, S, S), np.float32)
    for b in range(B):
        a = res.results[b]["attns"]  # (H, S, S)
        for h in range(H):
            attns[h * B + b] = a[h]
    return out.astype(np.float32), attns


# revision 5
# speedup vs baseline: 1.0357x; 1.0357x over previous
"""Trainium2 Bass kernel for nn_MultiHeadAttention (B=8, S=1024, D=1024, H=16).

Strategy: data-parallel over batch across the 8 NeuronCores (attention is
independent per batch element, so no collectives are needed). Each core runs an
identical Tile program on its batch slice:

  - QKV projections as bf16 matmuls (weights pre-concatenated per head on host,
    TEMPER folded into Wq; inputs fed pre-transposed so the contraction dim is
    on partitions).
  - Scores are computed twice per head - once natural (i on partitions) for the
    softmax/attns output, once transposed (j on partitions) to feed P@V without
    an on-chip transpose of P.
  - The boolean mask is applied as an additive -50 bias (via an
    identity-matmul accumulate into PSUM) on the natural path and as a {0,1}
    multiplier on the transposed path.
  - exp runs on the scalar engine with accum_out producing row sums for free;
    the softmax divide is a per-partition tensor_scalar; the transposed path
    stays unnormalized and the 1/r correction is applied to O = P@V via a
    DMA-broadcast of 1/r.
  - Output projection + residual + (unscaled) layernorm on-chip; the ln_g/ln_b
    affine is exact and applied on host (ones/zeros in practice).

Outputs: normalized attention probabilities per head (f32) and the layernormed
output (f32), gathered/reassembled on host.
"""

import numpy as np
import ml_dtypes

B, S, D = 8, 1024, 1024
H, DK, DV = 16, 64, 64
NT = S // 128
TEMPER = float(np.sqrt(D))
NCORES = 8
BF16 = ml_dtypes.bfloat16

# Set by test harness to capture a NTFF profile (requires the axon profile
# hook to be installed by the caller). Off by default for robustness.
PROFILE = False
LAST_EXEC_NS = None
LAST_RESULTS = None

_compiled_nc = None


def _build_nc():
    import concourse.bass as bass
    import concourse.mybir as mybir
    import concourse.tile as tile
    from concourse import bacc
    from concourse.masks import make_identity

    bf = mybir.dt.bfloat16
    f32 = mybir.dt.float32
    Exp = mybir.ActivationFunctionType.Exp
    Sqrt = mybir.ActivationFunctionType.Sqrt

    nc = bacc.Bacc("TRN2", target_bir_lowering=False, debug=False,
                   num_devices=NCORES)

    p_qt = nc.declare_dram_parameter("qt", [D, S], bf, isOutput=False)
    p_kt = nc.declare_dram_parameter("kt", [D, S], bf, isOutput=False)
    p_vt = nc.declare_dram_parameter("vt", [D, S], bf, isOutput=False)
    p_wq = nc.declare_dram_parameter("wq", [D, H * DK], bf, isOutput=False)
    p_wk = nc.declare_dram_parameter("wk", [D, H * DK], bf, isOutput=False)
    p_wv = nc.declare_dram_parameter("wv", [D, H * DV], bf, isOutput=False)
    p_pwt = nc.declare_dram_parameter("pwt", [H * DV, D], bf, isOutput=False)
    p_mb = nc.declare_dram_parameter("mb", [S, S], bf, isOutput=False)
    p_mft = nc.declare_dram_parameter("mft", [S, S], bf, isOutput=False)
    p_qres = nc.declare_dram_parameter("qres", [S, D], f32, isOutput=False)
    p_attn = nc.declare_dram_parameter("attns", [H, S, S], bf, isOutput=True)
    p_out = nc.declare_dram_parameter("outn", [S, D], f32, isOutput=True)
    rdram = nc.dram_tensor("rscratch", [NT, 2, S], f32)

    def t3(p):
        # DRAM (R, C) -> SBUF-shaped (128, R/128, C) view
        return p.ap().rearrange("(t p) c -> p t c", p=128)

    with tile.TileContext(nc) as tc:
        ident, _f0 = tc.tile([128, 128], bf, name="ident")
        make_identity(nc, ident)
        epst, _f1 = tc.tile([128, 1], f32, name="epst")
        nc.vector.memset(epst, 1e-5)
        QT, _f2 = tc.tile([128, NT, S], bf, name="QT")
        KT, _f3 = tc.tile([128, NT, S], bf, name="KT")
        VS, _f4 = tc.tile([128, NT, H * DV], bf, name="VS")
        MBs, _f5 = tc.tile([128, NT, S], bf, name="MBs")
        MFTs, _f6 = tc.tile([128, NT, S], bf, name="MFTs")
        OT, _f7 = tc.tile([128, NT, S], bf, name="OT")

        with tc.tile_pool(name="w2m", bufs=3) as wpool, \
             tc.tile_pool(name="pa", bufs=8) as papool, \
             tc.tile_pool(name="fx", bufs=2) as fxpool, \
             tc.tile_pool(name="sm", bufs=4) as smpool, \
             tc.tile_pool(name="psb", bufs=4, space="PSUM") as psb:

            nc.sync.dma_start(out=MBs, in_=t3(p_mb))
            nc.sync.dma_start(out=MFTs, in_=t3(p_mft))

            # ---- Stage P: projections ----
            # QT = (q @ Wq/TEMPER).T, KT = (k @ Wk).T  (head-dim on partitions)
            # VS = v @ Wv (natural: sequence on partitions)
            for src, wsrc, dst, transposed in ((p_qt, p_wq, QT, True),
                                               (p_kt, p_wk, KT, True),
                                               (p_vt, p_wv, VS, False)):
                x_t = wpool.tile([128, NT, S], bf, tag="w2m", name="x_t")
                w_t = wpool.tile([128, NT, S], bf, tag="w2m", name="w_t")
                nc.sync.dma_start(out=x_t, in_=t3(src))
                nc.sync.dma_start(out=w_t, in_=t3(wsrc))
                for m in range(NT):
                    ps = psb.tile([128, 1024], f32, tag="big", name="ps")
                    for nch in range(2):
                        nsl = slice(nch * 512, nch * 512 + 512)
                        for kt_i in range(NT):
                            if transposed:
                                lhs = w_t[:, kt_i, m * 128:(m + 1) * 128]
                                rhs = x_t[:, kt_i, nsl]
                            else:
                                lhs = x_t[:, kt_i, m * 128:(m + 1) * 128]
                                rhs = w_t[:, kt_i, nsl]
                            nc.tensor.matmul(ps[:, nsl], lhsT=lhs, rhs=rhs,
                                             start=(kt_i == 0),
                                             stop=(kt_i == NT - 1))
                    nc.vector.tensor_copy(out=dst[:, m, :], in_=ps)

            # ---- Stage A: attention, head pairs packed in the PE array ----
            for t2 in range(NT):
                puts = []
                rinvs = []
                for hh in range(2):
                    h = 2 * t2 + hh
                    hs = slice(hh * 64, hh * 64 + 64)
                    # path A: natural scores -> exp(+rowsum) -> normalize -> attns
                    racc = smpool.tile([128, NT], f32, tag="racc", name="racc")
                    pa_tiles = []
                    for it in range(NT):
                        psA = psb.tile([128, 1024], f32, tag="big", name="psA")
                        for jc in range(2):
                            jsl = slice(jc * 512, jc * 512 + 512)
                            nc.tensor.matmul(psA[:, jsl], lhsT=ident,
                                             rhs=MBs[:, it, jsl],
                                             start=True, stop=False)
                            nc.tensor.matmul(psA[:, jsl],
                                             lhsT=QT[hs, t2, it * 128:(it + 1) * 128],
                                             rhs=KT[hs, t2, jsl],
                                             start=False, stop=True,
                                             tile_position=(hh * 64, 0))
                        pa = papool.tile([128, 1024], bf, tag="pa", name="pa")
                        nc.scalar.activation(out=pa, in_=psA, func=Exp,
                                             accum_out=racc[:, it:it + 1])
                        pa_tiles.append(pa)
                    rinv = smpool.tile([128, NT], f32, tag="rinv", name="rinv")
                    nc.vector.reciprocal(out=rinv, in_=racc)
                    rinvs.append(rinv)
                    for it in range(NT):
                        nc.vector.tensor_scalar_mul(out=pa_tiles[it],
                                                    in0=pa_tiles[it],
                                                    scalar1=rinv[:, it:it + 1])
                        nc.gpsimd.dma_start(
                            out=p_attn.ap()[h, it * 128:(it + 1) * 128, :],
                            in_=pa_tiles[it])
                    # path B: transposed scores -> exp -> mask-mul (unnormalized)
                    put = wpool.tile([128, NT, S], bf, tag="w2m", name="put")
                    for jt in range(NT):
                        psB = psb.tile([128, 1024], f32, tag="big", name="psB")
                        for ic in range(2):
                            isl = slice(ic * 512, ic * 512 + 512)
                            nc.tensor.matmul(psB[:, isl],
                                             lhsT=KT[hs, t2, jt * 128:(jt + 1) * 128],
                                             rhs=QT[hs, t2, isl],
                                             start=True, stop=True,
                                             tile_position=(hh * 64, 0))
                        nc.scalar.activation(out=put[:, jt, :], in_=psB, func=Exp)
                        nc.vector.tensor_mul(out=put[:, jt, :],
                                             in0=put[:, jt, :],
                                             in1=MFTs[:, jt, :])
                    puts.append(put)
                # 1/r broadcast: write i-major to DRAM, read back replicated
                for hh in range(2):
                    nc.gpsimd.dma_start(
                        out=rdram.ap()[t2, hh].rearrange("(t p) -> p t", p=128),
                        in_=rinvs[hh])
                rrep = fxpool.tile([128, 1024], f32, tag="rrep", name="rrep")
                for hh in range(2):
                    src_ap = rdram.ap()[t2, hh]
                    b_ap = bass.AP(tensor=src_ap.tensor, offset=src_ap.offset,
                                   ap=[[0, 64]] + list(src_ap.ap))
                    nc.sync.dma_start(out=rrep[hh * 64:hh * 64 + 64, :], in_=b_ap)
                # P@V for the head pair (col-packed), then normalize rows by 1/r
                psO = psb.tile([128, 1024], f32, tag="big", name="psO")
                for ic in range(2):
                    isl = slice(ic * 512, ic * 512 + 512)
                    for jt in range(NT):
                        for hh in range(2):
                            h = 2 * t2 + hh
                            nc.tensor.matmul(psO[hh * 64:hh * 64 + 64, isl],
                                             lhsT=VS[:, jt, h * 64:(h + 1) * 64],
                                             rhs=puts[hh][:, jt, isl],
                                             start=(jt == 0), stop=(jt == NT - 1),
                                             tile_position=(0, hh * 64),
                                             skip_group_check=True)
                nc.vector.tensor_mul(out=OT[:, t2, :], in0=psO, in1=rrep)

            # ---- Stage F: projection + residual + layernorm ----
            pwt_t = wpool.tile([128, NT, D], bf, tag="w2m", name="pwt_t")
            nc.sync.dma_start(out=pwt_t, in_=t3(p_pwt))
            for st in range(NT):
                qr = fxpool.tile([128, 1024], f32, tag="qr", name="qr")
                nc.sync.dma_start(out=qr, in_=p_qres.ap()[st * 128:(st + 1) * 128, :])
                xm = fxpool.tile([128, 1024], f32, tag="xm", name="xm")
                psF = psb.tile([128, 1024], f32, tag="big", name="psF")
                for dc in range(2):
                    dsl = slice(dc * 512, dc * 512 + 512)
                    for ct in range(NT):
                        nc.tensor.matmul(psF[:, dsl],
                                         lhsT=OT[:, ct, st * 128:(st + 1) * 128],
                                         rhs=pwt_t[:, ct, dsl],
                                         start=(ct == 0), stop=(ct == NT - 1))
                nc.vector.tensor_add(out=xm, in0=psF, in1=qr)
                stats = smpool.tile([128, 2, 6], f32, tag="stats", name="stats")
                for c2 in range(2):
                    nc.vector.bn_stats(out=stats[:, c2, :],
                                       in_=xm[:, c2 * 512:(c2 + 1) * 512])
                mv = smpool.tile([128, 2], f32, tag="mv", name="mv")
                nc.vector.bn_aggr(out=mv, in_=stats)
                std = smpool.tile([128, 1], f32, tag="std", name="std")
                nc.scalar.activation(out=std, in_=mv[:, 1:2], func=Sqrt,
                                     bias=epst, scale=1.0)
                nc.vector.reciprocal(out=std, in_=std)
                nc.vector.tensor_scalar(out=xm, in0=xm,
                                        scalar1=mv[:, 0:1], scalar2=std,
                                        op0=mybir.AluOpType.subtract,
                                        op1=mybir.AluOpType.mult)
                nc.sync.dma_start(out=p_out.ap()[st * 128:(st + 1) * 128, :],
                                  in_=xm)
        for f in (_f7, _f6, _f5, _f4, _f3, _f2, _f1, _f0):
            f()
    nc.compile()
    return nc


def _get_nc():
    global _compiled_nc
    if _compiled_nc is None:
        _compiled_nc = _build_nc()
    return _compiled_nc


def kernel(q, k, v, attn_mask, w_qs, w_ks, w_vs, proj_w, proj_b, ln_g, ln_b):
    global LAST_EXEC_NS, LAST_RESULTS
    from concourse.bass_utils import run_bass_kernel_spmd

    q = np.asarray(q, np.float32)
    k = np.asarray(k, np.float32)
    v = np.asarray(v, np.float32)
    attn_mask = np.asarray(attn_mask, bool)
    w_qs = np.asarray(w_qs, np.float32)
    w_ks = np.asarray(w_ks, np.float32)
    w_vs = np.asarray(w_vs, np.float32)
    proj_w = np.asarray(proj_w, np.float32)
    proj_b = np.asarray(proj_b, np.float32)
    ln_g = np.asarray(ln_g, np.float32)
    ln_b = np.asarray(ln_b, np.float32)

    # host-side weight prep (shared across cores)
    wq = np.ascontiguousarray(
        (w_qs.transpose(1, 0, 2).reshape(D, H * DK) / TEMPER)).astype(BF16)
    wk = np.ascontiguousarray(
        w_ks.transpose(1, 0, 2).reshape(D, H * DK)).astype(BF16)
    wv = np.ascontiguousarray(
        w_vs.transpose(1, 0, 2).reshape(D, H * DV)).astype(BF16)
    pwt = np.ascontiguousarray(proj_w.T).astype(BF16)
    qres_all = q + proj_b  # fold the projection bias into the residual

    in_maps = []
    for b in range(B):
        in_maps.append(dict(
            qt=np.ascontiguousarray(q[b].T).astype(BF16),
            kt=np.ascontiguousarray(k[b].T).astype(BF16),
            vt=np.ascontiguousarray(v[b].T).astype(BF16),
            wq=wq, wk=wk, wv=wv, pwt=pwt,
            mb=np.where(attn_mask[b], np.float32(-50.0),
                        np.float32(0.0)).astype(BF16),
            mft=np.ascontiguousarray((~attn_mask[b]).T.astype(np.float32)).astype(BF16),
            qres=np.ascontiguousarray(qres_all[b]).astype(np.float32),
        ))

    nc = _get_nc()
    res = run_bass_kernel_spmd(nc, in_maps, core_ids=list(range(NCORES)),
                               trace=PROFILE)
    LAST_EXEC_NS = res.exec_time_ns
    LAST_RESULTS = res

    out = np.stack([res.results[b]["outn"] for b in range(B)], axis=0)
    out = out * ln_g + ln_b  # exact layernorm affine on host

    attns = np.empty((H * B, S, S), np.float32)
    for b in range(B):
        a = res.results[b]["attns"]  # (H, S, S) bf16
        for h in range(H):
            attns[h * B + b] = a[h].astype(np.float32)
    return out.astype(np.float32), attns
